# revision 58
# baseline (speedup 1.0000x reference)
"""Trainium2 Bass kernel for additive (Bahdanau) attention.

    c[b] = softmax_t( v_a . tanh(s[b] @ W_a + h[b] @ U_a) ) @ h[b]

Shapes (hardcoded): s [32,1024] f32, h [32,2048,1024] f32,
W_a [1024,512], U_a [1024,512], v_a [512]  ->  c [32,1024] f32.

Sharding: data-parallel over batch; 8 NeuronCores x 4 batches each.
W_a/U_a/v_a replicated. No cross-core communication.

v14 design (evolved from v3 baseline at 189.9us; now 169.7us,
rel_err 1.81e-2 vs the 2e-2 gate):
  - Main matmul in fp8(e4m3) DoubleRow mode: contracts d-chunk PAIRS,
    2x bf16 PE throughput (measured: 227.7ns for 256-deep x 512-col
    vs 225.9ns bf16 at 128-deep). U cast to fp8 by the DMA (SWDGE),
    hT8 produced by ScalarE/DVE casts from the transpose PSUM.
    Accuracy cost is ~1.8e-2 (measured, deterministic inputs), almost
    entirely e4m3 quantization of U and h; all other paths stay bf16.
  - ALL h supertiles are DMA-prefetched up front (14-deep ring + 2
    deferred loads): SWDGE (gpsimd) DMAs emitted later pick up a
    program-order barrier against every previously-emitted DVE op,
    which serialized the whole pipeline when loads were issued
    per-iteration.
  - c-path avoids both the 1x-rate DVE stt-with-accumulator and any
    bf16 hT copies: pT (p with t on partitions, via tiny PE matmuls on
    the replicated exp row) scales h natural per-partition on the DVE
    (tensor_scalar 2x + bf16 pair adds), and the PE contracts the 128
    t_lo partitions with a ones[128,32] stationary into c_st
    replicated on 2x32 partitions; batch epilogue is 3 small adds + a
    scale, and c[b] leaves as two contiguous 2KB DMA writes.
  - 3-stage software pipeline per iteration g: c-contract(g-2) |
    vdot/exp(g-1) | transposes+casts(g+1) | mains(g) | pT+hp(g-1).
    Every PE op consumes only results produced >= 1 iteration earlier,
    so no PE stall waits on the ACT exp or the DVE hp chain.
  - Warm-up dummy matmuls use a DVE-memset tile (no gpsimd identity
    dependency) so the PE is busy from ~0.3us and the HAM clock ramps;
    W/s/v load as f32 on the sync-HWDGE queue (parallel to the gpsimd
    cast queue) and are cast on the head-idle DVE.
  - Tail: the last supertile skips the DVE pair-adds (idle PE
    contracts all 4 hp tiles) and its c-contract runs in the same
    iteration as its hp chain.
"""

import numpy as np

import concourse.bacc as bacc
import concourse.tile as tile
import concourse.mybir as mybir
from concourse.bass_utils import run_bass_kernel_spmd
from concourse.masks import make_identity

N_CORES = 8
B, T, DH, DS, A = 32, 2048, 1024, 1024, 512
BPC = B // N_CORES          # batches per core
ST = 512                    # supertile rows (t)
NST = T // ST               # supertiles per batch
NTS = ST // 128             # 128-row chunks per supertile
NDC = DH // 128             # d chunks
NAC = A // 128              # a chunks
NGLOB = BPC * NST           # supertiles per core

F32 = mybir.dt.float32
BF16 = mybir.dt.bfloat16
FP8 = mybir.dt.float8e4
DR = mybir.MatmulPerfMode.DoubleRow
AF = mybir.ActivationFunctionType
MUL = mybir.AluOpType.mult

# number of d-chunk PAIRS of the main matmul done in fp8 DoubleRow mode
# (2x PE throughput). 0 = all bf16; 2 = chunks 0-3 fp8; 4 = all fp8.
N_FP8_PAIRS = 4
# trailing d-chunks whose c-partial runs as narrow PE matmuls against
# h natural (frees the DVE stt + the bf16 hT copy for those chunks)
C_PE = 2
N_STT = 8 - C_PE  # leading chunks on the DVE stt path
I32 = mybir.dt.int32


def build_nc():
    assert 2 * N_FP8_PAIRS == NDC, "only the full-fp8 mains path is wired"
    nc = bacc.Bacc("TRN2", target_bir_lowering=False, debug=False,
                   num_devices=N_CORES)
    s = nc.dram_tensor("s", [BPC, DS], F32, kind="ExternalInput").ap()
    h = nc.dram_tensor("h", [BPC, T, DH], F32, kind="ExternalInput").ap()
    W_a = nc.dram_tensor("W_a", [DS, A], F32, kind="ExternalInput").ap()
    U_a = nc.dram_tensor("U_a", [DH, A], F32, kind="ExternalInput").ap()
    v_a = nc.dram_tensor("v_a", [A], F32, kind="ExternalInput").ap()
    c = nc.dram_tensor("c", [BPC, DH], F32, kind="ExternalOutput").ap()

    with tile.TileContext(nc) as tc:
        with (
            tc.tile_pool(name="const", bufs=1) as const,
            tc.tile_pool(name="hpool", bufs=14) as hpool,
            tc.tile_pool(name="ht8pool", bufs=3) as ht8pool,
            tc.tile_pool(name="esbp", bufs=6) as esbp,
            tc.tile_pool(name="pexpp", bufs=2) as pexpp,
            tc.tile_pool(name="hppool", bufs=2) as hppool,
            tc.tile_pool(name="smalls", bufs=4) as smalls,
            tc.tile_pool(name="cres", bufs=2) as cres,
            tc.tile_pool(name="epool", bufs=3, space="PSUM") as epool,
            tc.tile_pool(name="tpsp", bufs=3, space="PSUM") as tpsp,
            tc.tile_pool(name="prp", bufs=2, space="PSUM") as prp,
        ):
            # ---- identity/ones BEFORE any dma_start: the gpsimd queue
            # is FIFO and the PE warm-up depends on the identity.
            ident = const.tile([128, 128], BF16, name="ident")
            make_identity(nc, ident)
            ones_row = const.tile([1, 128], BF16)
            nc.vector.memset(ones_row, 1.0)
            ones32 = const.tile([128, 32], BF16, name="ones32")
            nc.vector.memset(ones32, 1.0)
            junk = const.tile([128, 128], BF16, name="junk")
            nc.vector.memset(junk, 1.0)

            h_tiles = {}
            ht8_tiles = {}
            P8 = N_FP8_PAIRS

            warm_ps_holder = []

            def dummy_mms(n):
                for i in range(n):
                    nc.tensor.matmul(warm_ps_holder[0][:, 0:128], lhsT=junk,
                                     rhs=junk, start=True, stop=True,
                                     skip_group_check=True)

            def load_h(glob, split=False):
                # head tiles split per-ts so the first transposes unblock
                # per-quarter; steady state uses one 2MB DMA (max BW needs
                # >=1MiB per dma_start, and it is 1 gpsimd trigger not 4)
                b, st = glob // NST, glob % NST
                t = hpool.tile([128, NTS, DH], BF16, name=f"h_sb{glob}",
                               tag="h_sb")
                if split:
                    for ts in range(NTS):
                        nc.gpsimd.dma_start(
                            out=t[:, ts],
                            in_=h[b, ST * st + 128 * ts:
                                  ST * st + 128 * (ts + 1), :])
                else:
                    nc.gpsimd.dma_start(
                        out=t,
                        in_=h[b, ST * st:ST * (st + 1), :]
                        .rearrange("(ts p) d -> p ts d", p=128))
                h_tiles[glob] = t

            def xbar_h(glob):
                # PE transposes 32 [128,128] chunks -> bf16 PSUM.
                # bf16 hT kept only for the N_STT stt chunks, copied as
                # int32-bitcast on DVE (half the element count, bit-exact).
                # fp8 hT8 (all chunks, DoubleRow mains): groups 0,1 cast on
                # ScalarE, groups 2,3 via SWDGE casting DMA.
                h_sb = h_tiles[glob]
                ht8 = ht8pool.tile([128, NDC, NTS, 128], FP8,
                                   name=f"hT8_sb{glob}", tag="hT8_sb")
                for dcp in range(NDC // 2):
                    tps = tpsp.tile([128, 1024], BF16,
                                    name=f"tps{glob}_{dcp}", tag="tps")
                    for dch in range(2):
                        dc = 2 * dcp + dch
                        for ts in range(NTS):
                            nc.tensor.transpose(
                                tps[:, dch * 512 + ts * 128:
                                    dch * 512 + ts * 128 + 128],
                                h_sb[:, ts, 128 * dc:128 * (dc + 1)],
                                ident)
                    tview = tps.rearrange("p (dch ts t) -> p dch ts t",
                                          dch=2, ts=NTS)
                    # fp8 hT8 casts from PSUM: groups 0,1 on ScalarE,
                    # groups 2,3 on DVE
                    if dcp < 2:
                        nc.scalar.copy(ht8[:, 2 * dcp:2 * dcp + 2], tview)
                    else:
                        nc.vector.tensor_copy(ht8[:, 2 * dcp:2 * dcp + 2],
                                              tview)
                ht8_tiles[glob] = ht8

            # ---- DMA order. gpsimd/SWDGE q (casting): h0, U8, h1,
            # h2..h15 prefetch. sync/HWDGE q (parallel): W/s/v as f32,
            # cast on the head-idle DVE.
            s32 = const.tile([BPC, DS], F32)
            nc.sync.dma_start(out=s32, in_=s)
            v32 = const.tile([1, A], F32)
            nc.sync.dma_start(out=v32,
                              in_=v_a.rearrange("(o a) -> o a", o=1))
            W32 = const.tile([128, NDC, A], F32)
            nc.sync.dma_start(out=W32,
                              in_=W_a.rearrange("(dc p) a -> p dc a", p=128))
            s_sb = const.tile([BPC, DS], BF16)
            nc.vector.tensor_copy(s_sb, s32)
            v_row = const.tile([1, A], BF16)
            nc.vector.tensor_copy(v_row, v32)
            W_sb = const.tile([128, NDC, A], BF16)
            for dcp in range(4):
                nc.vector.tensor_copy(W_sb[:, 2 * dcp:2 * dcp + 2],
                                      W32[:, 2 * dcp:2 * dcp + 2])
            load_h(0, split=True)
            # U8 before h1: mains(0) needs it; xbar(1) runs later
            U8_sb = const.tile([128, 2 * P8, A], FP8)
            nc.gpsimd.dma_start(
                out=U8_sb,
                in_=U_a.rearrange("(dc p) a -> p dc a", p=128))
            load_h(1, split=True)
            # prefetch ALL remaining h supertiles now: SWDGE DMAs emitted
            # later pick up a program-order barrier against every DVE op
            # already emitted (the gpsimd cast path can't wait fine-
            # grained), which was pacing the whole pipeline. Emitted here,
            # they have no dependencies at all and stream at full HBM BW.
            for glob in range(2, NGLOB - 2):
                load_h(glob)

            # ---- PE warm-up (dummy matmuls, results unused): keeps the
            # PE busy while U/h0 land and engages the HAM fast clock.
            warm_ps = prp.tile([128, ST], F32, name="warm_ps", tag="prp")
            warm_ps_holder.append(warm_ps)
            dummy_mms(60)

            # ---- sT via PE transpose: [128 d_lo, dc, b] bf16
            sps = epool.tile([128, NDC, BPC], BF16, name="sps", tag="e_ps",
                             padded_shape=[128, NDC, 128])
            for dc in range(NDC):
                nc.tensor.transpose(
                    sps[:, dc, :],
                    s_sb[:, 128 * dc:128 * (dc + 1)],
                    ident[0:BPC, 0:BPC])
            sT_sb = const.tile([128, NDC, BPC], BF16)
            nc.vector.tensor_copy(sT_sb, sps)

            # ---- v_rep[a_lo, ac, j] = v[a] for all j (replicated cols)
            vr_ps = prp.tile([128, ST], F32, name="vr_ps", tag="prp")
            for ac in range(NAC):
                nc.tensor.matmul(vr_ps[:, 128 * ac:128 * (ac + 1)],
                                 lhsT=v_row[:, 128 * ac:128 * (ac + 1)],
                                 rhs=ones_row, start=True, stop=True,
                                 skip_group_check=True)
            v_rep = const.tile([128, NAC, 128], BF16)
            nc.vector.tensor_copy(v_rep, vr_ps)

            bias_sb = const.tile([128, NAC, BPC], F32)

            def emit_bias():
                for ac in range(NAC):
                    ws_ps = prp.tile([128, BPC], F32, name=f"ws_ps{ac}",
                                     tag="prp", padded_shape=[128, 512])
                    for dc in range(NDC):
                        nc.tensor.matmul(
                            ws_ps,
                            lhsT=W_sb[:, dc, 128 * ac:128 * (ac + 1)],
                            rhs=sT_sb[:, dc, :],
                            start=(dc == 0), stop=(dc == NDC - 1))
                    nc.vector.tensor_copy(bias_sb[:, ac, :], ws_ps)

            # ---- first supertile's transposes, ts-major: each arriving
            # h0 quarter unblocks 8 transposes (the dcp-major order needs
            # ALL quarters for every group, so the PE would trickle 2
            # transposes per quarter and the HAM re-throttles).  Dummy
            # matmuls pad the quarter gaps to hold the fast clock.
            h0_sb = h_tiles[0]
            ht8_0 = ht8pool.tile([128, NDC, NTS, 128], FP8,
                                 name="hT8_sb0", tag="hT8_sb")
            for ts in range(NTS):
                tq = epool.tile([128, NDC, 128], BF16, name=f"tq{ts}",
                                tag="e_ps")
                for dc in range(NDC):
                    nc.tensor.transpose(tq[:, dc, :],
                                        h0_sb[:, ts, 128 * dc:128 * (dc + 1)],
                                        ident)
                nc.scalar.copy(ht8_0[:, :, ts, :], tq)
                dummy_mms(20)
            ht8_tiles[0] = ht8_0

            # ---- main loop, one-iteration-deferred softmax/c chain ----
            e_tiles = {}    # glob -> list of 4 tanh'd e_sb tiles
            S4_tiles = {}
            cpart_tiles = {}
            p_exps = {}
            hp_tiles = {}

            def deferred_stage(g):
                # v-dots (PE), exp (ACT), c-partials (DVE hp chain + PE
                # ones-contract) for supertile g.
                b, st = g // NST, g % NST
                if st == 0:
                    S4_tiles[b] = smalls.tile([128, NST], F32,
                                              name=f"S4_{b}", tag="S4")
                    # st-level c partials: partitions 0:32 hold d 0:512,
                    # 32:64 hold d 512:1024
                    cpart_tiles[b] = cres.tile([64, NST, 512], BF16,
                                               name=f"cpart{b}", tag="cpart")
                e_sbs = e_tiles.pop(g)
                p_ps = prp.tile([128, ST], F32, name=f"p_ps{g}", tag="prp")
                for ac in range(NAC):
                    nc.tensor.matmul(p_ps, lhsT=v_rep[:, ac, :],
                                     rhs=e_sbs[ac],
                                     start=(ac == 0), stop=(ac == NAC - 1))
                p_exp = pexpp.tile([128, NTS, 128], BF16,
                                   name=f"p_exp{g}", tag="p_exp")
                nc.scalar.activation(p_exp, p_ps, AF.Exp,
                                     accum_out=S4_tiles[b][:, st:st + 1])
                p_exps[g] = p_exp

            def pt_hp_stage(g):
                # pT extraction (tiny PE matmuls on exp output) + the DVE
                # hp chain. Runs late in the iteration, after mains, so
                # the PE never waits on the ACT exp.
                p_exp = p_exps.pop(g)
                h_nat = h_tiles.pop(g)
                pT_ps = prp.tile([128, NTS], F32, name=f"pT_ps{g}",
                                 tag="prp", padded_shape=[128, 512])
                for ts in range(NTS):
                    nc.tensor.matmul(pT_ps[:, ts:ts + 1],
                                     lhsT=p_exp[0:1, ts, :],
                                     rhs=ones_row[:, 0:1],
                                     start=True, stop=True,
                                     skip_group_check=True)
                pT_sb = smalls.tile([128, NTS], F32, name=f"pT_sb{g}",
                                    tag="pT_sb")
                nc.vector.tensor_copy(pT_sb, pT_ps)
                # hp[t_lo, d] partial sums over the 4 t-chunks: 4
                # independent per-partition-scalar muls (h dependency ends
                # here) + 3 bf16 adds
                hps = []
                for ts in range(NTS):
                    hp = hppool.tile([128, DH], BF16, name=f"hp{g}_{ts}",
                                     tag=f"hp{ts}")
                    nc.vector.tensor_scalar_mul(hp, h_nat[:, ts, :],
                                                pT_sb[:, ts:ts + 1])
                    hps.append(hp)
                if g == NGLOB - 1:
                    # tail: skip the DVE adds; the idle PE contracts all
                    # four hp tiles (shortens the post-mains chain)
                    hp_tiles[g] = hps
                else:
                    nc.vector.tensor_add(hps[0], hps[0], hps[1])
                    nc.vector.tensor_add(hps[2], hps[2], hps[3])
                    nc.vector.tensor_add(hps[0], hps[0], hps[2])
                    hp_tiles[g] = hps[0]

            def c_stage(g):
                # PE ones-contract of hp(g) (ready since last iteration)
                # into c_st, replicated over 2x32 partitions
                b, st = g // NST, g % NST
                hp_a = hp_tiles.pop(g)
                hp_list = hp_a if isinstance(hp_a, list) else [hp_a]
                # c_ps lives in the prp ring (not epool): its WAR partners
                # (pT_ps readers two stages back) are always >=1 iteration
                # stale, and epool then holds only the 4-deep e_ps
                # rotation, freeing a bank for a third tps buffer
                c_ps = prp.tile([128, ST], F32, name=f"c_ps{g}",
                                tag="prp")
                for half in range(2):
                    for i, hp in enumerate(hp_list):
                        nc.tensor.matmul(c_ps[32 * half:32 * half + 32, :],
                                         lhsT=ones32,
                                         rhs=hp[:, 512 * half:
                                                512 * (half + 1)],
                                         start=(i == 0),
                                         stop=(i == len(hp_list) - 1),
                                         skip_group_check=True)
                nc.vector.tensor_copy(cpart_tiles[b][:, st, :],
                                      c_ps[0:64, :])
                if st == NST - 1:
                    batch_epilogue(b)

            def batch_epilogue(b):
                Ssum = smalls.tile([128, 1], F32, name=f"Ssum{b}", tag="Ssum")
                nc.vector.reduce_sum(Ssum, S4_tiles[b],
                                     axis=mybir.AxisListType.X)
                rS = smalls.tile([128, 1], F32, name=f"rS{b}", tag="rS")
                nc.vector.reciprocal(rS, Ssum)
                cp = cpart_tiles[b]
                s01 = cres.tile([64, 512], BF16, name=f"s01_{b}", tag="s01", bufs=1)
                nc.vector.tensor_add(s01, cp[:, 0, :], cp[:, 1, :])
                s23 = cres.tile([64, 512], BF16, name=f"s23_{b}", tag="s23", bufs=1)
                nc.vector.tensor_add(s23, cp[:, 2, :], cp[:, 3, :])
                csum = cres.tile([64, 512], F32, name=f"csum{b}", tag="csum", bufs=1)
                nc.vector.tensor_add(csum, s01, s23)
                c_fin = cres.tile([64, 512], F32, name=f"c_fin{b}",
                                  tag="c_fin")
                nc.vector.tensor_scalar_mul(c_fin, csum, rS[0:64])
                nc.sync.dma_start(out=c[b:b + 1, 0:512], in_=c_fin[0:1, :])
                nc.sync.dma_start(out=c[b:b + 1, 512:1024],
                                  in_=c_fin[32:33, :])

            dummy_mms(24)

            for g in range(NGLOB + 1):
                if g == 4:
                    # last two h tiles deferred so the 14-deep ring fits
                    # SBUF; their SWDGE DVE-barriers are long satisfied
                    load_h(NGLOB - 2)
                if g == NGLOB // 2:
                    load_h(NGLOB - 1)
                # c-contract of g-2: its hp chain finished last iteration
                if g >= 2:
                    c_stage(g - 2)
                # vdot+exp of g-1 (PE then ACT, pipelined behind mains)
                if 1 <= g <= NGLOB:
                    deferred_stage(g - 1)
                # transposes+casts for g+1 BEFORE mains(g): their ACT/DVE
                # casts then land an iteration ahead of mains(g+1)
                if 1 <= g and g + 1 < NGLOB:
                    xbar_h(g + 1)
                if g < NGLOB:
                    b = g // NST
                    hT8 = ht8_tiles.pop(g, None)
                    e_sbs = []
                    e_pss = []
                    for ac in range(NAC):
                        e_ps = epool.tile([128, ST], F32,
                                          name=f"e_ps{g}_{ac}", tag="e_ps")
                        for p in range(P8):
                            nc.tensor.matmul(
                                e_ps,
                                lhsT=U8_sb[:, 2 * p:2 * p + 2,
                                           128 * ac:128 * (ac + 1)],
                                rhs=hT8[:, 2 * p:2 * p + 2],
                                start=(p == 0),
                                stop=(2 * P8 == NDC and p == P8 - 1),
                                perf_mode=DR, skip_group_check=True)
                        e_pss.append(e_ps)
                        if g == 0:
                            # tanh(0,*) is emitted after emit_bias() below
                            # so the bias RAW dep is tracked; the bias
                            # matmuls then sit after mains(0) in the PE
                            # queue, where W_a has landed.
                            continue
                        e_sb = esbp.tile([128, ST], BF16,
                                         name=f"e_sb{g}_{ac}", tag="e_sb")
                        nc.scalar.activation(e_sb, e_ps, AF.Tanh,
                                             bias=bias_sb[:, ac, b:b + 1])
                        e_sbs.append(e_sb)
                    if g == 0:
                        emit_bias()
                        for ac in range(NAC):
                            e_sb = esbp.tile([128, ST], BF16,
                                             name=f"e_sb0_{ac}", tag="e_sb")
                            nc.scalar.activation(e_sb, e_pss[ac], AF.Tanh,
                                                 bias=bias_sb[:, ac, 0:1])
                            e_sbs.append(e_sb)
                    e_tiles[g] = e_sbs
                    if g == 0:
                        # xbar(1) stays after mains(0): h1 is still landing
                        xbar_h(1)
                # pT + hp of g-1 after mains(g): the PE pT matmuls come
                # after ~3.6us of mains, so the ACT exp(g-1) is done
                if 1 <= g <= NGLOB:
                    pt_hp_stage(g - 1)
                if g == NGLOB:
                    # tail: contract the final supertile immediately
                    c_stage(NGLOB - 1)

    nc.finalize()
    return nc


_NC_CACHE = None


def kernel(s, h, W_a, U_a, v_a):
    global _NC_CACHE
    if _NC_CACHE is None:
        _NC_CACHE = build_nc()
    nc = _NC_CACHE
    s = np.ascontiguousarray(s, dtype=np.float32)
    h = np.ascontiguousarray(h, dtype=np.float32)
    W_a = np.ascontiguousarray(W_a, dtype=np.float32)
    U_a = np.ascontiguousarray(U_a, dtype=np.float32)
    v_a = np.ascontiguousarray(v_a, dtype=np.float32)
    in_maps = [
        {"s": s[i * BPC:(i + 1) * BPC], "h": h[i * BPC:(i + 1) * BPC],
         "W_a": W_a, "U_a": U_a, "v_a": v_a}
        for i in range(N_CORES)
    ]
    res = run_bass_kernel_spmd(nc, in_maps, core_ids=list(range(N_CORES)))
    return np.concatenate([res.results[i]["c"] for i in range(N_CORES)], axis=0)



# revision 60
# speedup vs baseline: 1.1233x; 1.1233x over previous
"""Trainium2 Bass kernel for additive (Bahdanau) attention.

    c[b] = softmax_t( v_a . tanh(s[b] @ W_a + h[b] @ U_a) ) @ h[b]

Shapes (hardcoded): s [32,1024] f32, h [32,2048,1024] f32,
W_a [1024,512], U_a [1024,512], v_a [512]  ->  c [32,1024] f32.

Sharding: data-parallel over batch; 8 NeuronCores x 4 batches each.
W_a/U_a/v_a replicated. No cross-core communication.

v14 design (evolved from v3 baseline at 189.9us; now 169.7us,
rel_err 1.81e-2 vs the 2e-2 gate):
  - Main matmul in fp8(e4m3) DoubleRow mode: contracts d-chunk PAIRS,
    2x bf16 PE throughput (measured: 227.7ns for 256-deep x 512-col
    vs 225.9ns bf16 at 128-deep). U cast to fp8 by the DMA (SWDGE),
    hT8 produced by ScalarE/DVE casts from the transpose PSUM.
    Accuracy cost is ~1.8e-2 (measured, deterministic inputs), almost
    entirely e4m3 quantization of U and h; all other paths stay bf16.
  - ALL h supertiles are DMA-prefetched up front (14-deep ring + 2
    deferred loads): SWDGE (gpsimd) DMAs emitted later pick up a
    program-order barrier against every previously-emitted DVE op,
    which serialized the whole pipeline when loads were issued
    per-iteration.
  - c-path avoids both the 1x-rate DVE stt-with-accumulator and any
    bf16 hT copies: pT (p with t on partitions, via tiny PE matmuls on
    the replicated exp row) scales h natural per-partition on the DVE
    (tensor_scalar 2x + bf16 pair adds), and the PE contracts the 128
    t_lo partitions with a ones[128,32] stationary into c_st
    replicated on 2x32 partitions; batch epilogue is 3 small adds + a
    scale, and c[b] leaves as two contiguous 2KB DMA writes.
  - 3-stage software pipeline per iteration g: c-contract(g-2) |
    vdot/exp(g-1) | transposes+casts(g+1) | mains(g) | pT+hp(g-1).
    Every PE op consumes only results produced >= 1 iteration earlier,
    so no PE stall waits on the ACT exp or the DVE hp chain.
  - Warm-up dummy matmuls use a DVE-memset tile (no gpsimd identity
    dependency) so the PE is busy from ~0.3us and the HAM clock ramps;
    W/s/v load as f32 on the sync-HWDGE queue (parallel to the gpsimd
    cast queue) and are cast on the head-idle DVE.
  - Tail: the last supertile skips the DVE pair-adds (idle PE
    contracts all 4 hp tiles) and its c-contract runs in the same
    iteration as its hp chain.
"""

import numpy as np

import concourse.bacc as bacc
import concourse.tile as tile
import concourse.mybir as mybir
from concourse.bass_utils import run_bass_kernel_spmd
from concourse.masks import make_identity

N_CORES = 8
B, T, DH, DS, A = 32, 2048, 1024, 1024, 512
BPC = B // N_CORES          # batches per core
ST = 512                    # supertile rows (t)
NST = T // ST               # supertiles per batch
NTS = ST // 128             # 128-row chunks per supertile
NDC = DH // 128             # d chunks
NAC = A // 128              # a chunks
NGLOB = BPC * NST           # supertiles per core

F32 = mybir.dt.float32
BF16 = mybir.dt.bfloat16
FP8 = mybir.dt.float8e4
DR = mybir.MatmulPerfMode.DoubleRow
AF = mybir.ActivationFunctionType
MUL = mybir.AluOpType.mult

# number of d-chunk PAIRS of the main matmul done in fp8 DoubleRow mode
# (2x PE throughput). 0 = all bf16; 2 = chunks 0-3 fp8; 4 = all fp8.
N_FP8_PAIRS = 4
# trailing d-chunks whose c-partial runs as narrow PE matmuls against
# h natural (frees the DVE stt + the bf16 hT copy for those chunks)
C_PE = 2
N_STT = 8 - C_PE  # leading chunks on the DVE stt path
I32 = mybir.dt.int32


def build_nc():
    assert 2 * N_FP8_PAIRS == NDC, "only the full-fp8 mains path is wired"
    nc = bacc.Bacc("TRN2", target_bir_lowering=False, debug=False,
                   num_devices=N_CORES)
    s = nc.dram_tensor("s", [BPC, DS], F32, kind="ExternalInput").ap()
    h = nc.dram_tensor("h", [BPC, T, DH], F32, kind="ExternalInput").ap()
    W_a = nc.dram_tensor("W_a", [DS, A], F32, kind="ExternalInput").ap()
    U_a = nc.dram_tensor("U_a", [DH, A], F32, kind="ExternalInput").ap()
    v_a = nc.dram_tensor("v_a", [A], F32, kind="ExternalInput").ap()
    c = nc.dram_tensor("c", [BPC, DH], F32, kind="ExternalOutput").ap()

    with tile.TileContext(nc) as tc:
        with (
            tc.tile_pool(name="const", bufs=1) as const,
            tc.tile_pool(name="hpool", bufs=14) as hpool,
            tc.tile_pool(name="ht8pool", bufs=3) as ht8pool,
            tc.tile_pool(name="esbp", bufs=6) as esbp,
            tc.tile_pool(name="pexpp", bufs=2) as pexpp,
            tc.tile_pool(name="hppool", bufs=2) as hppool,
            tc.tile_pool(name="smalls", bufs=4) as smalls,
            tc.tile_pool(name="cres", bufs=2) as cres,
            tc.tile_pool(name="epool", bufs=4, space="PSUM") as epool,
            tc.tile_pool(name="tpsp", bufs=2, space="PSUM") as tpsp,
            tc.tile_pool(name="prp", bufs=2, space="PSUM") as prp,
        ):
            # ---- identity/ones BEFORE any dma_start: the gpsimd queue
            # is FIFO and the PE warm-up depends on the identity.
            ident = const.tile([128, 128], BF16, name="ident")
            make_identity(nc, ident)
            ones_row = const.tile([1, 128], BF16)
            nc.vector.memset(ones_row, 1.0)
            ones32 = const.tile([128, 32], BF16, name="ones32")
            nc.vector.memset(ones32, 1.0)
            junk = const.tile([128, 128], BF16, name="junk")
            nc.vector.memset(junk, 1.0)

            h_tiles = {}
            ht8_tiles = {}
            P8 = N_FP8_PAIRS

            warm_ps_holder = []

            def dummy_mms(n):
                for i in range(n):
                    nc.tensor.matmul(warm_ps_holder[0][:, 0:128], lhsT=junk,
                                     rhs=junk, start=True, stop=True,
                                     skip_group_check=True)

            def load_h(glob, split=False):
                # head tiles split per-ts so the first transposes unblock
                # per-quarter; steady state uses one 2MB DMA (max BW needs
                # >=1MiB per dma_start, and it is 1 gpsimd trigger not 4)
                b, st = glob // NST, glob % NST
                t = hpool.tile([128, NTS, DH], BF16, name=f"h_sb{glob}",
                               tag="h_sb")
                if split:
                    for ts in range(NTS):
                        nc.gpsimd.dma_start(
                            out=t[:, ts],
                            in_=h[b, ST * st + 128 * ts:
                                  ST * st + 128 * (ts + 1), :])
                else:
                    nc.gpsimd.dma_start(
                        out=t,
                        in_=h[b, ST * st:ST * (st + 1), :]
                        .rearrange("(ts p) d -> p ts d", p=128))
                h_tiles[glob] = t

            def xbar_h(glob):
                # PE transposes 32 [128,128] chunks -> bf16 PSUM.
                # bf16 hT kept only for the N_STT stt chunks, copied as
                # int32-bitcast on DVE (half the element count, bit-exact).
                # fp8 hT8 (all chunks, DoubleRow mains): groups 0,1 cast on
                # ScalarE, groups 2,3 via SWDGE casting DMA.
                h_sb = h_tiles[glob]
                ht8 = ht8pool.tile([128, NDC, NTS, 128], FP8,
                                   name=f"hT8_sb{glob}", tag="hT8_sb")
                for dcp in range(NDC // 2):
                    tps = tpsp.tile([128, 1024], BF16,
                                    name=f"tps{glob}_{dcp}", tag="tps")
                    for dch in range(2):
                        dc = 2 * dcp + dch
                        for ts in range(NTS):
                            nc.tensor.transpose(
                                tps[:, dch * 512 + ts * 128:
                                    dch * 512 + ts * 128 + 128],
                                h_sb[:, ts, 128 * dc:128 * (dc + 1)],
                                ident)
                    tview = tps.rearrange("p (dch ts t) -> p dch ts t",
                                          dch=2, ts=NTS)
                    # fp8 hT8 casts from PSUM: groups 0,1 on DVE (which
                    # reaches them ~2us before ScalarE clears exp, so the
                    # tps banks recycle for transposes dcp2/3 sooner),
                    # groups 2,3 on ScalarE
                    if dcp < 2:
                        nc.vector.tensor_copy(ht8[:, 2 * dcp:2 * dcp + 2],
                                              tview)
                    else:
                        nc.scalar.copy(ht8[:, 2 * dcp:2 * dcp + 2], tview)
                ht8_tiles[glob] = ht8

            # ---- DMA order. gpsimd/SWDGE q (casting): h0, U8, h1,
            # h2..h15 prefetch. sync/HWDGE q (parallel): W/s/v as f32,
            # cast on the head-idle DVE.
            s32 = const.tile([BPC, DS], F32)
            nc.sync.dma_start(out=s32, in_=s)
            v32 = const.tile([1, A], F32)
            nc.sync.dma_start(out=v32,
                              in_=v_a.rearrange("(o a) -> o a", o=1))
            W32 = const.tile([128, NDC, A], F32)
            nc.sync.dma_start(out=W32,
                              in_=W_a.rearrange("(dc p) a -> p dc a", p=128))
            s_sb = const.tile([BPC, DS], BF16)
            nc.vector.tensor_copy(s_sb, s32)
            v_row = const.tile([1, A], BF16)
            nc.vector.tensor_copy(v_row, v32)
            W_sb = const.tile([128, NDC, A], BF16)
            for dcp in range(4):
                nc.vector.tensor_copy(W_sb[:, 2 * dcp:2 * dcp + 2],
                                      W32[:, 2 * dcp:2 * dcp + 2])
            load_h(0, split=True)
            # U8 before h1: mains(0) needs it; xbar(1) runs later
            U8_sb = const.tile([128, 2 * P8, A], FP8)
            nc.gpsimd.dma_start(
                out=U8_sb,
                in_=U_a.rearrange("(dc p) a -> p dc a", p=128))
            load_h(1, split=True)
            # prefetch ALL remaining h supertiles now: SWDGE DMAs emitted
            # later pick up a program-order barrier against every DVE op
            # already emitted (the gpsimd cast path can't wait fine-
            # grained), which was pacing the whole pipeline. Emitted here,
            # they have no dependencies at all and stream at full HBM BW.
            for glob in range(2, NGLOB - 2):
                load_h(glob)

            # ---- PE warm-up (dummy matmuls, results unused): keeps the
            # PE busy while U/h0 land and engages the HAM fast clock.
            warm_ps = prp.tile([128, ST], F32, name="warm_ps", tag="prp")
            warm_ps_holder.append(warm_ps)
            dummy_mms(60)

            # ---- sT via PE transpose: [128 d_lo, dc, b] bf16
            sps = epool.tile([128, NDC, BPC], BF16, name="sps", tag="e_ps",
                             padded_shape=[128, NDC, 128])
            for dc in range(NDC):
                nc.tensor.transpose(
                    sps[:, dc, :],
                    s_sb[:, 128 * dc:128 * (dc + 1)],
                    ident[0:BPC, 0:BPC])
            sT_sb = const.tile([128, NDC, BPC], BF16)
            nc.vector.tensor_copy(sT_sb, sps)

            # ---- v_rep[a_lo, ac, j] = v[a] for all j (replicated cols)
            vr_ps = prp.tile([128, ST], F32, name="vr_ps", tag="prp")
            for ac in range(NAC):
                nc.tensor.matmul(vr_ps[:, 128 * ac:128 * (ac + 1)],
                                 lhsT=v_row[:, 128 * ac:128 * (ac + 1)],
                                 rhs=ones_row, start=True, stop=True,
                                 skip_group_check=True)
            v_rep = const.tile([128, NAC, 128], BF16)
            nc.vector.tensor_copy(v_rep, vr_ps)

            bias_sb = const.tile([128, NAC, BPC], F32)

            def emit_bias():
                for ac in range(NAC):
                    ws_ps = prp.tile([128, BPC], F32, name=f"ws_ps{ac}",
                                     tag="prp", padded_shape=[128, 512])
                    for dc in range(NDC):
                        nc.tensor.matmul(
                            ws_ps,
                            lhsT=W_sb[:, dc, 128 * ac:128 * (ac + 1)],
                            rhs=sT_sb[:, dc, :],
                            start=(dc == 0), stop=(dc == NDC - 1))
                    nc.vector.tensor_copy(bias_sb[:, ac, :], ws_ps)

            # ---- first supertile's transposes, ts-major: each arriving
            # h0 quarter unblocks 8 transposes (the dcp-major order needs
            # ALL quarters for every group, so the PE would trickle 2
            # transposes per quarter and the HAM re-throttles).  Dummy
            # matmuls pad the quarter gaps to hold the fast clock.
            h0_sb = h_tiles[0]
            ht8_0 = ht8pool.tile([128, NDC, NTS, 128], FP8,
                                 name="hT8_sb0", tag="hT8_sb")
            for ts in range(NTS):
                tq = epool.tile([128, NDC, 128], BF16, name=f"tq{ts}",
                                tag="e_ps")
                for dc in range(NDC):
                    nc.tensor.transpose(tq[:, dc, :],
                                        h0_sb[:, ts, 128 * dc:128 * (dc + 1)],
                                        ident)
                nc.scalar.copy(ht8_0[:, :, ts, :], tq)
                dummy_mms(20)
            ht8_tiles[0] = ht8_0

            # ---- main loop, one-iteration-deferred softmax/c chain ----
            e_tiles = {}    # glob -> list of 4 tanh'd e_sb tiles
            S4_tiles = {}
            cpart_tiles = {}
            p_exps = {}
            hp_tiles = {}

            def deferred_stage(g):
                # v-dots (PE), exp (ACT), c-partials (DVE hp chain + PE
                # ones-contract) for supertile g.
                b, st = g // NST, g % NST
                if st == 0:
                    S4_tiles[b] = smalls.tile([128, NST], F32,
                                              name=f"S4_{b}", tag="S4")
                    # st-level c partials: partitions 0:32 hold d 0:512,
                    # 32:64 hold d 512:1024
                    cpart_tiles[b] = cres.tile([64, NST, 512], BF16,
                                               name=f"cpart{b}", tag="cpart")
                e_sbs = e_tiles.pop(g)
                p_ps = prp.tile([128, ST], F32, name=f"p_ps{g}", tag="prp")
                for ac in range(NAC):
                    nc.tensor.matmul(p_ps, lhsT=v_rep[:, ac, :],
                                     rhs=e_sbs[ac],
                                     start=(ac == 0), stop=(ac == NAC - 1))
                p_exp = pexpp.tile([128, NTS, 128], BF16,
                                   name=f"p_exp{g}", tag="p_exp")
                nc.scalar.activation(p_exp, p_ps, AF.Exp,
                                     accum_out=S4_tiles[b][:, st:st + 1])
                p_exps[g] = p_exp

            def pt_hp_stage(g):
                # pT extraction (tiny PE matmuls on exp output) + the DVE
                # hp chain. Runs late in the iteration, after mains, so
                # the PE never waits on the ACT exp.
                p_exp = p_exps.pop(g)
                h_nat = h_tiles.pop(g)
                pT_ps = prp.tile([128, NTS], F32, name=f"pT_ps{g}",
                                 tag="prp", padded_shape=[128, 512])
                for ts in range(NTS):
                    nc.tensor.matmul(pT_ps[:, ts:ts + 1],
                                     lhsT=p_exp[0:1, ts, :],
                                     rhs=ones_row[:, 0:1],
                                     start=True, stop=True,
                                     skip_group_check=True)
                pT_sb = smalls.tile([128, NTS], F32, name=f"pT_sb{g}",
                                    tag="pT_sb")
                nc.vector.tensor_copy(pT_sb, pT_ps)
                # hp[t_lo, d] partial sums over the 4 t-chunks: 4
                # independent per-partition-scalar muls (h dependency ends
                # here) + 3 bf16 adds
                hps = []
                for ts in range(NTS):
                    hp = hppool.tile([128, DH], BF16, name=f"hp{g}_{ts}",
                                     tag=f"hp{ts}")
                    nc.vector.tensor_scalar_mul(hp, h_nat[:, ts, :],
                                                pT_sb[:, ts:ts + 1])
                    hps.append(hp)
                if g == NGLOB - 1:
                    # tail: skip the DVE adds; the idle PE contracts all
                    # four hp tiles (shortens the post-mains chain)
                    hp_tiles[g] = hps
                else:
                    nc.vector.tensor_add(hps[0], hps[0], hps[1])
                    nc.vector.tensor_add(hps[2], hps[2], hps[3])
                    nc.vector.tensor_add(hps[0], hps[0], hps[2])
                    hp_tiles[g] = hps[0]

            def c_stage(g):
                # PE ones-contract of hp(g) (ready since last iteration)
                # into c_st, replicated over 2x32 partitions
                b, st = g // NST, g % NST
                hp_a = hp_tiles.pop(g)
                hp_list = hp_a if isinstance(hp_a, list) else [hp_a]
                c_ps = epool.tile([128, ST], F32, name=f"c_ps{g}",
                                  tag="e_ps")
                for half in range(2):
                    for i, hp in enumerate(hp_list):
                        nc.tensor.matmul(c_ps[32 * half:32 * half + 32, :],
                                         lhsT=ones32,
                                         rhs=hp[:, 512 * half:
                                                512 * (half + 1)],
                                         start=(i == 0),
                                         stop=(i == len(hp_list) - 1),
                                         skip_group_check=True)
                nc.vector.tensor_copy(cpart_tiles[b][:, st, :],
                                      c_ps[0:64, :])
                if st == NST - 1:
                    batch_epilogue(b)

            def batch_epilogue(b):
                Ssum = smalls.tile([128, 1], F32, name=f"Ssum{b}", tag="Ssum")
                nc.vector.reduce_sum(Ssum, S4_tiles[b],
                                     axis=mybir.AxisListType.X)
                rS = smalls.tile([128, 1], F32, name=f"rS{b}", tag="rS")
                nc.vector.reciprocal(rS, Ssum)
                cp = cpart_tiles[b]
                s01 = cres.tile([64, 512], BF16, name=f"s01_{b}", tag="s01", bufs=1)
                nc.vector.tensor_add(s01, cp[:, 0, :], cp[:, 1, :])
                s23 = cres.tile([64, 512], BF16, name=f"s23_{b}", tag="s23", bufs=1)
                nc.vector.tensor_add(s23, cp[:, 2, :], cp[:, 3, :])
                csum = cres.tile([64, 512], F32, name=f"csum{b}", tag="csum", bufs=1)
                nc.vector.tensor_add(csum, s01, s23)
                c_fin = cres.tile([64, 512], F32, name=f"c_fin{b}",
                                  tag="c_fin")
                nc.vector.tensor_scalar_mul(c_fin, csum, rS[0:64])
                nc.sync.dma_start(out=c[b:b + 1, 0:512], in_=c_fin[0:1, :])
                nc.sync.dma_start(out=c[b:b + 1, 512:1024],
                                  in_=c_fin[32:33, :])

            dummy_mms(24)

            for g in range(NGLOB + 1):
                if g == 4:
                    # last two h tiles deferred so the 14-deep ring fits
                    # SBUF; their SWDGE DVE-barriers are long satisfied
                    load_h(NGLOB - 2)
                if g == NGLOB // 2:
                    load_h(NGLOB - 1)
                # c-contract of g-2: its hp chain finished last iteration
                if g >= 2:
                    c_stage(g - 2)
                # vdot+exp of g-1 (PE then ACT, pipelined behind mains)
                if 1 <= g <= NGLOB:
                    deferred_stage(g - 1)
                # transposes+casts for g+1 BEFORE mains(g): their ACT/DVE
                # casts then land an iteration ahead of mains(g+1)
                if 1 <= g and g + 1 < NGLOB:
                    xbar_h(g + 1)
                if g < NGLOB:
                    b = g // NST
                    hT8 = ht8_tiles.pop(g, None)
                    e_sbs = []
                    e_pss = []
                    for ac in range(NAC):
                        e_ps = epool.tile([128, ST], F32,
                                          name=f"e_ps{g}_{ac}", tag="e_ps")
                        for p in range(P8):
                            nc.tensor.matmul(
                                e_ps,
                                lhsT=U8_sb[:, 2 * p:2 * p + 2,
                                           128 * ac:128 * (ac + 1)],
                                rhs=hT8[:, 2 * p:2 * p + 2],
                                start=(p == 0),
                                stop=(2 * P8 == NDC and p == P8 - 1),
                                perf_mode=DR, skip_group_check=True)
                        e_pss.append(e_ps)
                        if g == 0:
                            # tanh(0,*) is emitted after emit_bias() below
                            # so the bias RAW dep is tracked; the bias
                            # matmuls then sit after mains(0) in the PE
                            # queue, where W_a has landed.
                            continue
                        e_sb = esbp.tile([128, ST], BF16,
                                         name=f"e_sb{g}_{ac}", tag="e_sb")
                        nc.scalar.activation(e_sb, e_ps, AF.Tanh,
                                             bias=bias_sb[:, ac, b:b + 1])
                        e_sbs.append(e_sb)
                    if g == 0:
                        emit_bias()
                        for ac in range(NAC):
                            e_sb = esbp.tile([128, ST], BF16,
                                             name=f"e_sb0_{ac}", tag="e_sb")
                            nc.scalar.activation(e_sb, e_pss[ac], AF.Tanh,
                                                 bias=bias_sb[:, ac, 0:1])
                            e_sbs.append(e_sb)
                    e_tiles[g] = e_sbs
                    if g == 0:
                        # xbar(1) stays after mains(0): h1 is still landing
                        xbar_h(1)
                # pT + hp of g-1 after mains(g): the PE pT matmuls come
                # after ~3.6us of mains, so the ACT exp(g-1) is done
                if 1 <= g <= NGLOB:
                    pt_hp_stage(g - 1)
                if g == NGLOB:
                    # tail: contract the final supertile immediately
                    c_stage(NGLOB - 1)

    nc.finalize()
    return nc


_NC_CACHE = None


def kernel(s, h, W_a, U_a, v_a):
    global _NC_CACHE
    if _NC_CACHE is None:
        _NC_CACHE = build_nc()
    nc = _NC_CACHE
    s = np.ascontiguousarray(s, dtype=np.float32)
    h = np.ascontiguousarray(h, dtype=np.float32)
    W_a = np.ascontiguousarray(W_a, dtype=np.float32)
    U_a = np.ascontiguousarray(U_a, dtype=np.float32)
    v_a = np.ascontiguousarray(v_a, dtype=np.float32)
    in_maps = [
        {"s": s[i * BPC:(i + 1) * BPC], "h": h[i * BPC:(i + 1) * BPC],
         "W_a": W_a, "U_a": U_a, "v_a": v_a}
        for i in range(N_CORES)
    ]
    res = run_bass_kernel_spmd(nc, in_maps, core_ids=list(range(N_CORES)))
    return np.concatenate([res.results[i]["c"] for i in range(N_CORES)], axis=0)



# revision 61
# speedup vs baseline: 1.1444x; 1.0188x over previous
"""Trainium2 Bass kernel for additive (Bahdanau) attention.

    c[b] = softmax_t( v_a . tanh(s[b] @ W_a + h[b] @ U_a) ) @ h[b]

Shapes (hardcoded): s [32,1024] f32, h [32,2048,1024] f32,
W_a [1024,512], U_a [1024,512], v_a [512]  ->  c [32,1024] f32.

Sharding: data-parallel over batch; 8 NeuronCores x 4 batches each.
W_a/U_a/v_a replicated. No cross-core communication.

v14 design (evolved from v3 baseline at 189.9us; now 169.7us,
rel_err 1.81e-2 vs the 2e-2 gate):
  - Main matmul in fp8(e4m3) DoubleRow mode: contracts d-chunk PAIRS,
    2x bf16 PE throughput (measured: 227.7ns for 256-deep x 512-col
    vs 225.9ns bf16 at 128-deep). U cast to fp8 by the DMA (SWDGE),
    hT8 produced by ScalarE/DVE casts from the transpose PSUM.
    Accuracy cost is ~1.8e-2 (measured, deterministic inputs), almost
    entirely e4m3 quantization of U and h; all other paths stay bf16.
  - ALL h supertiles are DMA-prefetched up front (14-deep ring + 2
    deferred loads): SWDGE (gpsimd) DMAs emitted later pick up a
    program-order barrier against every previously-emitted DVE op,
    which serialized the whole pipeline when loads were issued
    per-iteration.
  - c-path avoids both the 1x-rate DVE stt-with-accumulator and any
    bf16 hT copies: pT (p with t on partitions, via tiny PE matmuls on
    the replicated exp row) scales h natural per-partition on the DVE
    (tensor_scalar 2x + bf16 pair adds), and the PE contracts the 128
    t_lo partitions with a ones[128,32] stationary into c_st
    replicated on 2x32 partitions; batch epilogue is 3 small adds + a
    scale, and c[b] leaves as two contiguous 2KB DMA writes.
  - 3-stage software pipeline per iteration g: c-contract(g-2) |
    vdot/exp(g-1) | transposes+casts(g+1) | mains(g) | pT+hp(g-1).
    Every PE op consumes only results produced >= 1 iteration earlier,
    so no PE stall waits on the ACT exp or the DVE hp chain.
  - Warm-up dummy matmuls use a DVE-memset tile (no gpsimd identity
    dependency) so the PE is busy from ~0.3us and the HAM clock ramps;
    W/s/v load as f32 on the sync-HWDGE queue (parallel to the gpsimd
    cast queue) and are cast on the head-idle DVE.
  - Tail: the last supertile skips the DVE pair-adds (idle PE
    contracts all 4 hp tiles) and its c-contract runs in the same
    iteration as its hp chain.
"""

import numpy as np

import concourse.bacc as bacc
import concourse.tile as tile
import concourse.mybir as mybir
from concourse.bass_utils import run_bass_kernel_spmd
from concourse.masks import make_identity

N_CORES = 8
B, T, DH, DS, A = 32, 2048, 1024, 1024, 512
BPC = B // N_CORES          # batches per core
ST = 512                    # supertile rows (t)
NST = T // ST               # supertiles per batch
NTS = ST // 128             # 128-row chunks per supertile
NDC = DH // 128             # d chunks
NAC = A // 128              # a chunks
NGLOB = BPC * NST           # supertiles per core

F32 = mybir.dt.float32
BF16 = mybir.dt.bfloat16
FP8 = mybir.dt.float8e4
DR = mybir.MatmulPerfMode.DoubleRow
AF = mybir.ActivationFunctionType
MUL = mybir.AluOpType.mult

# number of d-chunk PAIRS of the main matmul done in fp8 DoubleRow mode
# (2x PE throughput). 0 = all bf16; 2 = chunks 0-3 fp8; 4 = all fp8.
N_FP8_PAIRS = 4
# trailing d-chunks whose c-partial runs as narrow PE matmuls against
# h natural (frees the DVE stt + the bf16 hT copy for those chunks)
C_PE = 2
N_STT = 8 - C_PE  # leading chunks on the DVE stt path
I32 = mybir.dt.int32


def build_nc():
    assert 2 * N_FP8_PAIRS == NDC, "only the full-fp8 mains path is wired"
    nc = bacc.Bacc("TRN2", target_bir_lowering=False, debug=False,
                   num_devices=N_CORES)
    s = nc.dram_tensor("s", [BPC, DS], F32, kind="ExternalInput").ap()
    h = nc.dram_tensor("h", [BPC, T, DH], F32, kind="ExternalInput").ap()
    W_a = nc.dram_tensor("W_a", [DS, A], F32, kind="ExternalInput").ap()
    U_a = nc.dram_tensor("U_a", [DH, A], F32, kind="ExternalInput").ap()
    v_a = nc.dram_tensor("v_a", [A], F32, kind="ExternalInput").ap()
    c = nc.dram_tensor("c", [BPC, DH], F32, kind="ExternalOutput").ap()

    with tile.TileContext(nc) as tc:
        with (
            tc.tile_pool(name="const", bufs=1) as const,
            tc.tile_pool(name="hpool", bufs=14) as hpool,
            tc.tile_pool(name="ht8pool", bufs=3) as ht8pool,
            tc.tile_pool(name="esbp", bufs=6) as esbp,
            tc.tile_pool(name="pexpp", bufs=2) as pexpp,
            tc.tile_pool(name="hppool", bufs=2) as hppool,
            tc.tile_pool(name="smalls", bufs=4) as smalls,
            tc.tile_pool(name="cres", bufs=2) as cres,
            tc.tile_pool(name="epool", bufs=4, space="PSUM") as epool,
            tc.tile_pool(name="tpsp", bufs=2, space="PSUM") as tpsp,
            tc.tile_pool(name="prp", bufs=2, space="PSUM") as prp,
        ):
            # ---- identity/ones BEFORE any dma_start: the gpsimd queue
            # is FIFO and the PE warm-up depends on the identity.
            ident = const.tile([128, 128], BF16, name="ident")
            make_identity(nc, ident)
            ones_row = const.tile([1, 128], BF16)
            nc.vector.memset(ones_row, 1.0)
            ones32 = const.tile([128, 32], BF16, name="ones32")
            nc.vector.memset(ones32, 1.0)
            junk = const.tile([128, 128], BF16, name="junk")
            nc.vector.memset(junk, 1.0)

            h_tiles = {}
            ht8_tiles = {}
            P8 = N_FP8_PAIRS

            warm_ps_holder = []

            def dummy_mms(n):
                for i in range(n):
                    nc.tensor.matmul(warm_ps_holder[0][:, 0:128], lhsT=junk,
                                     rhs=junk, start=True, stop=True,
                                     skip_group_check=True)

            def load_h(glob, split=False):
                # head tiles split per-ts so the first transposes unblock
                # per-quarter; steady state uses one 2MB DMA (max BW needs
                # >=1MiB per dma_start, and it is 1 gpsimd trigger not 4)
                b, st = glob // NST, glob % NST
                t = hpool.tile([128, NTS, DH], BF16, name=f"h_sb{glob}",
                               tag="h_sb")
                if split:
                    for ts in range(NTS):
                        nc.gpsimd.dma_start(
                            out=t[:, ts],
                            in_=h[b, ST * st + 128 * ts:
                                  ST * st + 128 * (ts + 1), :])
                else:
                    nc.gpsimd.dma_start(
                        out=t,
                        in_=h[b, ST * st:ST * (st + 1), :]
                        .rearrange("(ts p) d -> p ts d", p=128))
                h_tiles[glob] = t

            def xbar_h(glob):
                # PE transposes 32 [128,128] chunks -> bf16 PSUM.
                # bf16 hT kept only for the N_STT stt chunks, copied as
                # int32-bitcast on DVE (half the element count, bit-exact).
                # fp8 hT8 (all chunks, DoubleRow mains): groups 0,1 cast on
                # ScalarE, groups 2,3 via SWDGE casting DMA.
                h_sb = h_tiles[glob]
                ht8 = ht8pool.tile([128, NDC, NTS, 128], FP8,
                                   name=f"hT8_sb{glob}", tag="hT8_sb")
                for dcp in range(NDC // 2):
                    tps = tpsp.tile([128, 1024], BF16,
                                    name=f"tps{glob}_{dcp}", tag="tps")
                    for dch in range(2):
                        dc = 2 * dcp + dch
                        for ts in range(NTS):
                            nc.tensor.transpose(
                                tps[:, dch * 512 + ts * 128:
                                    dch * 512 + ts * 128 + 128],
                                h_sb[:, ts, 128 * dc:128 * (dc + 1)],
                                ident)
                    tview = tps.rearrange("p (dch ts t) -> p dch ts t",
                                          dch=2, ts=NTS)
                    # fp8 hT8 casts from PSUM: groups 0,1 on ScalarE,
                    # groups 2,3 on DVE
                    if dcp < 2:
                        nc.scalar.copy(ht8[:, 2 * dcp:2 * dcp + 2], tview)
                    else:
                        nc.vector.tensor_copy(ht8[:, 2 * dcp:2 * dcp + 2],
                                              tview)
                ht8_tiles[glob] = ht8

            # ---- DMA order. gpsimd/SWDGE q (casting): h0, U8, h1,
            # h2..h15 prefetch. sync/HWDGE q (parallel): W/s/v as f32,
            # cast on the head-idle DVE.
            s32 = const.tile([BPC, DS], F32)
            nc.sync.dma_start(out=s32, in_=s)
            v32 = const.tile([1, A], F32)
            nc.sync.dma_start(out=v32,
                              in_=v_a.rearrange("(o a) -> o a", o=1))
            W32 = const.tile([128, NDC, A], F32)
            nc.sync.dma_start(out=W32,
                              in_=W_a.rearrange("(dc p) a -> p dc a", p=128))
            s_sb = const.tile([BPC, DS], BF16)
            nc.vector.tensor_copy(s_sb, s32)
            v_row = const.tile([1, A], BF16)
            nc.vector.tensor_copy(v_row, v32)
            W_sb = const.tile([128, NDC, A], BF16)
            for dcp in range(4):
                nc.vector.tensor_copy(W_sb[:, 2 * dcp:2 * dcp + 2],
                                      W32[:, 2 * dcp:2 * dcp + 2])
            load_h(0, split=True)
            # U8 before h1: mains(0) needs it; xbar(1) runs later
            U8_sb = const.tile([128, 2 * P8, A], FP8)
            nc.gpsimd.dma_start(
                out=U8_sb,
                in_=U_a.rearrange("(dc p) a -> p dc a", p=128))
            load_h(1, split=True)
            # prefetch ALL remaining h supertiles now: SWDGE DMAs emitted
            # later pick up a program-order barrier against every DVE op
            # already emitted (the gpsimd cast path can't wait fine-
            # grained), which was pacing the whole pipeline. Emitted here,
            # they have no dependencies at all and stream at full HBM BW.
            for glob in range(2, NGLOB - 2):
                load_h(glob)

            # ---- PE warm-up (dummy matmuls, results unused): keeps the
            # PE busy while U/h0 land and engages the HAM fast clock.
            warm_ps = prp.tile([128, ST], F32, name="warm_ps", tag="prp")
            warm_ps_holder.append(warm_ps)
            dummy_mms(60)

            # ---- sT via PE transpose: [128 d_lo, dc, b] bf16
            sps = epool.tile([128, NDC, BPC], BF16, name="sps", tag="e_ps",
                             padded_shape=[128, NDC, 128])
            for dc in range(NDC):
                nc.tensor.transpose(
                    sps[:, dc, :],
                    s_sb[:, 128 * dc:128 * (dc + 1)],
                    ident[0:BPC, 0:BPC])
            sT_sb = const.tile([128, NDC, BPC], BF16)
            nc.vector.tensor_copy(sT_sb, sps)

            # ---- v_rep[a_lo, ac, j] = v[a] for all j (replicated cols)
            vr_ps = prp.tile([128, ST], F32, name="vr_ps", tag="prp")
            for ac in range(NAC):
                nc.tensor.matmul(vr_ps[:, 128 * ac:128 * (ac + 1)],
                                 lhsT=v_row[:, 128 * ac:128 * (ac + 1)],
                                 rhs=ones_row, start=True, stop=True,
                                 skip_group_check=True)
            v_rep = const.tile([128, NAC, 128], BF16)
            nc.vector.tensor_copy(v_rep, vr_ps)

            bias_sb = const.tile([128, NAC, BPC], F32)

            def emit_bias():
                for ac in range(NAC):
                    ws_ps = prp.tile([128, BPC], F32, name=f"ws_ps{ac}",
                                     tag="prp", padded_shape=[128, 512])
                    for dc in range(NDC):
                        nc.tensor.matmul(
                            ws_ps,
                            lhsT=W_sb[:, dc, 128 * ac:128 * (ac + 1)],
                            rhs=sT_sb[:, dc, :],
                            start=(dc == 0), stop=(dc == NDC - 1))
                    nc.vector.tensor_copy(bias_sb[:, ac, :], ws_ps)

            # ---- first supertile's transposes, ts-major: each arriving
            # h0 quarter unblocks 8 transposes (the dcp-major order needs
            # ALL quarters for every group, so the PE would trickle 2
            # transposes per quarter and the HAM re-throttles).  Dummy
            # matmuls pad the quarter gaps to hold the fast clock.
            h0_sb = h_tiles[0]
            ht8_0 = ht8pool.tile([128, NDC, NTS, 128], FP8,
                                 name="hT8_sb0", tag="hT8_sb")
            for ts in range(NTS):
                tq = epool.tile([128, NDC, 128], BF16, name=f"tq{ts}",
                                tag="e_ps")
                for dc in range(NDC):
                    nc.tensor.transpose(tq[:, dc, :],
                                        h0_sb[:, ts, 128 * dc:128 * (dc + 1)],
                                        ident)
                nc.scalar.copy(ht8_0[:, :, ts, :], tq)
                dummy_mms(20)
            ht8_tiles[0] = ht8_0

            # ---- main loop, one-iteration-deferred softmax/c chain ----
            e_tiles = {}    # glob -> list of 4 tanh'd e_sb tiles
            S4_tiles = {}
            cpart_tiles = {}
            p_exps = {}
            hp_tiles = {}

            def deferred_stage(g):
                # v-dots (PE), exp (ACT), c-partials (DVE hp chain + PE
                # ones-contract) for supertile g.
                b, st = g // NST, g % NST
                if st == 0:
                    S4_tiles[b] = smalls.tile([128, NST], F32,
                                              name=f"S4_{b}", tag="S4")
                    # st-level c partials: partitions 0:32 hold d 0:512,
                    # 32:64 hold d 512:1024
                    cpart_tiles[b] = cres.tile([64, NST, 512], BF16,
                                               name=f"cpart{b}", tag="cpart")
                e_sbs = e_tiles.pop(g)
                p_ps = prp.tile([128, ST], F32, name=f"p_ps{g}", tag="prp")
                for ac in range(NAC):
                    nc.tensor.matmul(p_ps, lhsT=v_rep[:, ac, :],
                                     rhs=e_sbs[ac],
                                     start=(ac == 0), stop=(ac == NAC - 1))
                p_exp = pexpp.tile([128, NTS, 128], BF16,
                                   name=f"p_exp{g}", tag="p_exp")
                nc.scalar.activation(p_exp, p_ps, AF.Exp,
                                     accum_out=S4_tiles[b][:, st:st + 1])
                p_exps[g] = p_exp

            def pt_hp_stage(g):
                # pT extraction (tiny PE matmuls on exp output) + the DVE
                # hp chain. Runs late in the iteration, after mains, so
                # the PE never waits on the ACT exp.
                p_exp = p_exps.pop(g)
                h_nat = h_tiles.pop(g)
                pT_ps = prp.tile([128, NTS], F32, name=f"pT_ps{g}",
                                 tag="prp", padded_shape=[128, 512])
                for ts in range(NTS):
                    nc.tensor.matmul(pT_ps[:, ts:ts + 1],
                                     lhsT=p_exp[0:1, ts, :],
                                     rhs=ones_row[:, 0:1],
                                     start=True, stop=True,
                                     skip_group_check=True)
                pT_sb = smalls.tile([128, NTS], F32, name=f"pT_sb{g}",
                                    tag="pT_sb")
                nc.vector.tensor_copy(pT_sb, pT_ps)
                # hp[t_lo, d] partial sums over the 4 t-chunks: 4
                # independent per-partition-scalar muls (h dependency ends
                # here) + 3 bf16 adds
                hps = []
                for ts in range(NTS):
                    hp = hppool.tile([128, DH], BF16, name=f"hp{g}_{ts}",
                                     tag=f"hp{ts}")
                    nc.vector.tensor_scalar_mul(hp, h_nat[:, ts, :],
                                                pT_sb[:, ts:ts + 1])
                    hps.append(hp)
                if g == NGLOB - 1:
                    # tail: skip the DVE adds; the idle PE contracts all
                    # four hp tiles (shortens the post-mains chain)
                    hp_tiles[g] = hps
                else:
                    nc.vector.tensor_add(hps[0], hps[0], hps[1])
                    nc.vector.tensor_add(hps[2], hps[2], hps[3])
                    nc.vector.tensor_add(hps[0], hps[0], hps[2])
                    hp_tiles[g] = hps[0]

            def c_stage(g):
                # PE ones-contract of hp(g) (ready since last iteration)
                # into c_st, replicated over 2x32 partitions
                b, st = g // NST, g % NST
                hp_a = hp_tiles.pop(g)
                hp_list = hp_a if isinstance(hp_a, list) else [hp_a]
                c_ps = epool.tile([128, ST], F32, name=f"c_ps{g}",
                                  tag="e_ps")
                for half in range(2):
                    for i, hp in enumerate(hp_list):
                        nc.tensor.matmul(c_ps[32 * half:32 * half + 32, :],
                                         lhsT=ones32,
                                         rhs=hp[:, 512 * half:
                                                512 * (half + 1)],
                                         start=(i == 0),
                                         stop=(i == len(hp_list) - 1),
                                         skip_group_check=True)
                nc.vector.tensor_copy(cpart_tiles[b][:, st, :],
                                      c_ps[0:64, :])
                if st == NST - 1:
                    batch_epilogue(b)

            def batch_epilogue(b):
                Ssum = smalls.tile([128, 1], F32, name=f"Ssum{b}", tag="Ssum")
                nc.vector.reduce_sum(Ssum, S4_tiles[b],
                                     axis=mybir.AxisListType.X)
                rS = smalls.tile([128, 1], F32, name=f"rS{b}", tag="rS")
                nc.vector.reciprocal(rS, Ssum)
                cp = cpart_tiles[b]
                s01 = cres.tile([64, 512], BF16, name=f"s01_{b}", tag="s01", bufs=1)
                nc.vector.tensor_add(s01, cp[:, 0, :], cp[:, 1, :])
                s23 = cres.tile([64, 512], BF16, name=f"s23_{b}", tag="s23", bufs=1)
                nc.vector.tensor_add(s23, cp[:, 2, :], cp[:, 3, :])
                csum = cres.tile([64, 512], F32, name=f"csum{b}", tag="csum", bufs=1)
                nc.vector.tensor_add(csum, s01, s23)
                c_fin = cres.tile([64, 512], F32, name=f"c_fin{b}",
                                  tag="c_fin")
                nc.vector.tensor_scalar_mul(c_fin, csum, rS[0:64])
                nc.sync.dma_start(out=c[b:b + 1, 0:512], in_=c_fin[0:1, :])
                nc.sync.dma_start(out=c[b:b + 1, 512:1024],
                                  in_=c_fin[32:33, :])

            dummy_mms(24)

            for g in range(NGLOB + 1):
                if g == 4:
                    # last two h tiles deferred so the 14-deep ring fits
                    # SBUF; their SWDGE DVE-barriers are long satisfied
                    load_h(NGLOB - 2)
                if g == NGLOB // 2:
                    load_h(NGLOB - 1)
                # c-contract of g-2: its hp chain finished last iteration
                if g >= 2:
                    c_stage(g - 2)
                # vdot+exp of g-1 (PE then ACT, pipelined behind mains)
                if 1 <= g <= NGLOB:
                    deferred_stage(g - 1)
                # transposes+casts for g+1 BEFORE mains(g): their ACT/DVE
                # casts then land an iteration ahead of mains(g+1)
                if 1 <= g and g + 1 < NGLOB:
                    xbar_h(g + 1)
                if g < NGLOB:
                    b = g // NST
                    hT8 = ht8_tiles.pop(g, None)
                    e_sbs = []
                    e_pss = []
                    for ac in range(NAC):
                        e_ps = epool.tile([128, ST], F32,
                                          name=f"e_ps{g}_{ac}", tag="e_ps")
                        for p in range(P8):
                            nc.tensor.matmul(
                                e_ps,
                                lhsT=U8_sb[:, 2 * p:2 * p + 2,
                                           128 * ac:128 * (ac + 1)],
                                rhs=hT8[:, 2 * p:2 * p + 2],
                                start=(p == 0),
                                stop=(2 * P8 == NDC and p == P8 - 1),
                                perf_mode=DR, skip_group_check=True)
                        e_pss.append(e_ps)
                        if g == 0:
                            # tanh(0,*) is emitted after emit_bias() below
                            # so the bias RAW dep is tracked; the bias
                            # matmuls then sit after mains(0) in the PE
                            # queue, where W_a has landed.
                            continue
                        e_sb = esbp.tile([128, ST], BF16,
                                         name=f"e_sb{g}_{ac}", tag="e_sb")
                        nc.scalar.activation(e_sb, e_ps, AF.Tanh,
                                             bias=bias_sb[:, ac, b:b + 1])
                        e_sbs.append(e_sb)
                    if g == 0:
                        emit_bias()
                        for ac in range(NAC):
                            e_sb = esbp.tile([128, ST], BF16,
                                             name=f"e_sb0_{ac}", tag="e_sb")
                            nc.scalar.activation(e_sb, e_pss[ac], AF.Tanh,
                                                 bias=bias_sb[:, ac, 0:1])
                            e_sbs.append(e_sb)
                    e_tiles[g] = e_sbs
                    if g == 0:
                        # xbar(1) stays after mains(0): h1 is still landing
                        xbar_h(1)
                # pT + hp of g-1 after mains(g): the PE pT matmuls come
                # after ~3.6us of mains, so the ACT exp(g-1) is done
                if 1 <= g <= NGLOB:
                    pt_hp_stage(g - 1)
                if g == NGLOB:
                    # tail: contract the final supertile immediately
                    c_stage(NGLOB - 1)

    nc.finalize()
    return nc


_NC_CACHE = None


def kernel(s, h, W_a, U_a, v_a):
    global _NC_CACHE
    if _NC_CACHE is None:
        _NC_CACHE = build_nc()
    nc = _NC_CACHE
    s = np.ascontiguousarray(s, dtype=np.float32)
    h = np.ascontiguousarray(h, dtype=np.float32)
    W_a = np.ascontiguousarray(W_a, dtype=np.float32)
    U_a = np.ascontiguousarray(U_a, dtype=np.float32)
    v_a = np.ascontiguousarray(v_a, dtype=np.float32)
    in_maps = [
        {"s": s[i * BPC:(i + 1) * BPC], "h": h[i * BPC:(i + 1) * BPC],
         "W_a": W_a, "U_a": U_a, "v_a": v_a}
        for i in range(N_CORES)
    ]
    res = run_bass_kernel_spmd(nc, in_maps, core_ids=list(range(N_CORES)))
    return np.concatenate([res.results[i]["c"] for i in range(N_CORES)], axis=0)

